# revision 4
# baseline (speedup 1.0000x reference)
"""Trainium2 Bass kernel for nn_ConvBlock (conv1d x3 + per-subject BN + GELU).

Sharding: data-parallel over batch across 8 NeuronCores (32 items/core).
Per-subject BN stats are reduced across cores with an in-kernel AllReduce
of (sum, sumsq) per (subject, channel); counts are host-known constants.

Self-contained: shapes hardcoded, no sibling imports.
"""

import os
import sys
import types

import numpy as np

# ---------------------------------------------------------------- constants
B, CIN, COUT, T = 256, 271, 320, 512
S = 4  # subjects
NCORES = 8
BSH = B // NCORES  # 32 items per core
EPS = 1e-5

# channel tiling (partition dim is 128)
KT0 = [(0, 128), (128, 256), (256, CIN)]  # conv0 contraction tiles (271)
KT = [(0, 128), (128, 256), (256, COUT)]  # conv1/2 contraction tiles (320)
CT = [(0, 128), (128, 256), (256, COUT)]  # output-channel tiles (320)

_F32 = None  # filled lazily (mybir.dt.float32)


def _install_ntff_hook():
    """Optionally enable NTFF profiling under axon (for tracing only)."""
    try:
        if "antenv.axon_hooks" not in sys.modules:
            import antenv  # noqa: F401

            mod = types.ModuleType("antenv.axon_hooks")
            _hook = [None]
            mod.set_axon_ntff_profile_hook = lambda h: _hook.__setitem__(0, h)
            mod.get_axon_ntff_profile_hook = lambda: _hook[0]
            sys.modules["antenv.axon_hooks"] = mod
            antenv.axon_hooks = mod
        from antenv.axon_hooks import (
            get_axon_ntff_profile_hook,
            set_axon_ntff_profile_hook,
        )

        if get_axon_ntff_profile_hook() is None:
            from trn_agent_boot.trn_boot import _ntff_profile_via_ctypes

            set_axon_ntff_profile_hook(
                _ntff_profile_via_ctypes("/opt/axon/libaxon_pjrt.so")
            )
    except Exception:
        pass


def _split_multi_waits(nc, mybir):
    """This env's walrus accepts one sync-wait per instruction: hoist extras
    onto separate same-engine nops placed just before the instruction."""
    for f in nc.m.functions:
        for bb in f.blocks:
            insts = list(bb.instructions)
            out = []
            changed = False
            for inst in insts:
                si = inst.sync_info
                if si is not None and si.on_wait and len(si.on_wait) > 1:
                    waits = list(si.on_wait)
                    for w in waits[:-1]:
                        d = mybir.InstNoOp(
                            name=nc.get_next_instruction_name(), ins=[], outs=[]
                        )
                        d.engine = inst.engine
                        d.sync_info = mybir.SyncInfo(on_wait=[w], on_update=[])
                        nc.register_instruction(d)
                        out.append(d)
                    inst.sync_info = mybir.SyncInfo(
                        on_wait=[waits[-1]], on_update=list(si.on_update or [])
                    )
                    changed = True
                out.append(inst)
            if changed:
                bb.instructions[:] = out


def _build_program():
    import concourse.bass as bass
    import concourse.mybir as mybir
    from concourse import tile

    F32 = mybir.dt.float32
    F32R = mybir.dt.float32r
    ADD = mybir.AluOpType.add
    MULT = mybir.AluOpType.mult
    SUB = mybir.AluOpType.subtract
    GELU = mybir.ActivationFunctionType.Gelu
    SQRT = mybir.ActivationFunctionType.Sqrt

    nc = bass.Bass("TRN2", target_bir_lowering=False, debug=False, num_devices=NCORES)

    # ---------------- I/O ----------------
    Xd = nc.dram_tensor("xsh", [BSH, CIN, T], F32, kind="ExternalInput").ap()
    wts = {}
    for s_i, cin in ((0, CIN), (1, COUT), (2, COUT)):
        for tap in range(3):
            wts[(s_i, tap)] = nc.dram_tensor(
                f"w{s_i}t{tap}", [cin, COUT], F32, kind="ExternalInput"
            ).ap()
    masksd = nc.dram_tensor("masks", [S, 128, BSH], F32, kind="ExternalInput").ap()
    invcd = nc.dram_tensor("invc", [128, S], F32, kind="ExternalInput").ap()
    gcmd = [
        nc.dram_tensor(f"gcm{s_i}", [COUT, S], F32, kind="ExternalInput").ap()
        for s_i in range(3)
    ]
    becmd = [
        nc.dram_tensor(f"becm{s_i}", [COUT, S], F32, kind="ExternalInput").ap()
        for s_i in range(3)
    ]
    OUTd = nc.dram_tensor("out", [BSH, COUT, T], F32, kind="ExternalOutput").ap()

    # DRAM scratch arenas (per-stage activations) + collective bounces
    Yd = [
        nc.dram_tensor(f"y{s_i}", [BSH, COUT, T], F32).ap() for s_i in range(3)
    ]
    ccin = [nc.dram_tensor(f"ccin{s_i}", [128, 24], F32).ap() for s_i in range(3)]
    ccout = [nc.dram_tensor(f"ccout{s_i}", [128, 24], F32).ap() for s_i in range(3)]

    with tile.TileContext(nc) as tc:
        with (
            tc.tile_pool(name="consts", bufs=1) as cpool,
            tc.tile_pool(name="wstag", bufs=2) as wstag,
            tc.tile_pool(name="zr", bufs=6) as zpool,
            tc.tile_pool(name="yin", bufs=6) as yinpool,
            tc.tile_pool(name="ynew", bufs=6) as ynpool,
            tc.tile_pool(name="sq", bufs=2) as sqpool,
            tc.tile_pool(name="isums", bufs=12) as ispool,
            tc.tile_pool(name="small", bufs=8) as smpool,
            tc.tile_pool(name="scsh", bufs=12) as scpool,
            tc.tile_pool(name="sctmp", bufs=4) as sctpool,
            tc.tile_pool(name="psum", bufs=4, space="PSUM") as pspool,
        ):
            # ---------------- load constants ----------------
            mask_t = []
            for s in range(S):
                mt = cpool.tile([128, BSH], F32, name=f"mask{s}")
                nc.sync.dma_start(mt[:], masksd[s])
                mask_t.append(mt)
            invc_t = cpool.tile([128, S], F32, name="invct")
            nc.sync.dma_start(invc_t[:], invcd[:])
            gcm_t = []  # [stage][ct] -> [128,4]
            becm_t = []
            for s_i in range(3):
                gl, bl = [], []
                for ci, (c0, c1) in enumerate(CT):
                    m = c1 - c0
                    g = cpool.tile([128, S], F32, name=f"g{s_i}_{ci}")
                    bt = cpool.tile([128, S], F32, name=f"b{s_i}_{ci}")
                    nc.sync.dma_start(g[:m, :], gcmd[s_i][c0:c1, :])
                    nc.sync.dma_start(bt[:m, :], becmd[s_i][c0:c1, :])
                    gl.append(g)
                    bl.append(bt)
                gcm_t.append(gl)
                becm_t.append(bl)

            # weights -> f32r tiles
            wr = {}  # (stage, kt, tap) -> [128, COUT] f32r tile
            for s_i in range(3):
                ktiles = KT0 if s_i == 0 else KT
                for ki, (k0, k1) in enumerate(ktiles):
                    ksz = k1 - k0
                    for tap in range(3):
                        stg = wstag.tile([128, COUT], F32, name="wstg")
                        nc.sync.dma_start(stg[:ksz, :], wts[(s_i, tap)][k0:k1, :])
                        wt = cpool.tile(
                            [128, COUT], F32R, name=f"wr{s_i}_{ki}_{tap}"
                        )
                        nc.vector.tensor_copy(wt[:ksz, :], stg[:ksz, :])
                        wr[(s_i, ki, tap)] = wt

            # per-item bn-apply scale/shift tiles, per stage
            SC = [None, None, None]  # stage -> [ct] -> [128, BSH]
            SH = [None, None, None]

            def conv_stage(s_i):
                """One full stage: (apply prev bn+gelu ->) conv -> sums; then
                stats allreduce -> per-item scale/shift for next apply."""
                ktiles = KT0 if s_i == 0 else KT
                i1 = [ispool.tile([128, BSH], F32, name=f"i1_{s_i}_{c}") for c in range(3)]
                i2 = [ispool.tile([128, BSH], F32, name=f"i2_{s_i}_{c}") for c in range(3)]

                for b in range(BSH):
                    # ---- produce conv input z (f32r) ----
                    zr = []
                    for ki, (k0, k1) in enumerate(ktiles):
                        ksz = k1 - k0
                        if s_i == 0:
                            xst = yinpool.tile([128, T], F32, name="yin")
                            nc.sync.dma_start(xst[:ksz, :], Xd[b, k0:k1, :])
                            zt = zpool.tile([128, T + 4], F32R, name="zr")
                            nc.vector.memset(zt[:ksz, 0:2].bitcast(F32), 0.0)
                            nc.vector.memset(zt[:ksz, T + 2 : T + 4].bitcast(F32), 0.0)
                            nc.vector.tensor_copy(zt[:ksz, 2 : T + 2], xst[:ksz, :])
                        else:
                            yin = yinpool.tile([128, T], F32, name="yin")
                            nc.sync.dma_start(yin[:ksz, :], Yd[s_i - 1][b, k0:k1, :])
                            zt = zpool.tile([128, T + 4], F32R, name="zr")
                            nc.vector.memset(zt[:ksz, 0:2].bitcast(F32), 0.0)
                            nc.vector.memset(zt[:ksz, T + 2 : T + 4].bitcast(F32), 0.0)
                            nc.scalar.activation(
                                zt[:ksz, 2 : T + 2],
                                yin[:ksz, :],
                                GELU,
                                bias=SH[s_i - 1][ki][:ksz, b : b + 1],
                                scale=SC[s_i - 1][ki][:ksz, b : b + 1],
                            )
                        zr.append(zt)

                    # ---- conv: 3 couttiles x (3 ktiles x 3 taps) ----
                    for ci, (c0, c1) in enumerate(CT):
                        m = c1 - c0
                        ps = pspool.tile([128, T], F32, name="ps")
                        first = True
                        n_mm = len(ktiles) * 3
                        done = 0
                        for ki, (k0, k1) in enumerate(ktiles):
                            ksz = k1 - k0
                            for tap in (1, 0, 2):
                                w = wr[(s_i, ki, tap)][:ksz, c0:c1]
                                off = 2 + (tap - 1)  # 1 / 2 / 3
                                r_ap = zr[ki][:ksz, off : off + T]
                                done += 1
                                nc.tensor.matmul(
                                    ps[:m, 0:T],
                                    w,
                                    r_ap,
                                    start=first,
                                    stop=(done == n_mm),
                                    skip_group_check=not first,
                                )
                                first = False

                        # ---- y = psum (+ residual z); per-item channel sums ----
                        yt = ynpool.tile([128, T], F32, name="ynew")
                        if s_i == 0:
                            nc.vector.tensor_scalar(
                                out=yt[:m, :],
                                in0=ps[:m, :],
                                scalar1=1.0,
                                scalar2=0.0,
                                op0=MULT,
                                op1=ADD,
                                accum_out=i1[ci][:m, b : b + 1],
                            )
                        else:
                            nc.vector.scalar_tensor_tensor(
                                out=yt[:m, :],
                                in0=ps[:m, :],
                                scalar=0.0,
                                in1=zr[ci][:m, 2 : T + 2].bitcast(F32),
                                op0=ADD,
                                op1=ADD,
                                accum_out=i1[ci][:m, b : b + 1],
                            )
                        sq = sqpool.tile([128, T], F32, name="sq")
                        nc.vector.scalar_tensor_tensor(
                            out=sq[:m, :],
                            in0=yt[:m, :],
                            scalar=1.0,
                            in1=yt[:m, :],
                            op0=MULT,
                            op1=MULT,
                            accum_out=i2[ci][:m, b : b + 1],
                        )
                        nc.sync.dma_start(Yd[s_i][b, c0:c1, :], yt[:m, :])

                # ---------------- stats: mask-reduce + AllReduce ----------------
                cc = smpool.tile([128, 24], F32, name=f"cc{s_i}")
                scr = sctpool.tile([128, BSH], F32, name="scr")
                for ci in range(3):
                    for s in range(S):
                        nc.vector.scalar_tensor_tensor(
                            out=scr[:, :],
                            in0=i1[ci][:, :],
                            scalar=1.0,
                            in1=mask_t[s][:, :],
                            op0=MULT,
                            op1=MULT,
                            accum_out=cc[:, ci * 4 + s : ci * 4 + s + 1],
                        )
                        nc.vector.scalar_tensor_tensor(
                            out=scr[:, :],
                            in0=i2[ci][:, :],
                            scalar=1.0,
                            in1=mask_t[s][:, :],
                            op0=MULT,
                            op1=MULT,
                            accum_out=cc[:, 12 + ci * 4 + s : 12 + ci * 4 + s + 1],
                        )
                nc.sync.dma_start(ccin[s_i][:, :], cc[:, :])
                nc.gpsimd.collective_compute(
                    "AllReduce",
                    ADD,
                    replica_groups=[list(range(NCORES))],
                    ins=[ccin[s_i][:, :]],
                    outs=[ccout[s_i][:, :]],
                )
                gsb = smpool.tile([128, 24], F32, name=f"gsb{s_i}")
                nc.sync.dma_start(gsb[:, :], ccout[s_i][:, :])

                # ---------------- scale/shift per (ct, subject) ----------------
                SCs, SHs = [], []
                for ci, (c0, c1) in enumerate(CT):
                    m = c1 - c0
                    g1 = gsb[:, ci * 4 : ci * 4 + 4]
                    g2 = gsb[:, 12 + ci * 4 : 12 + ci * 4 + 4]
                    mean = smpool.tile([128, S], F32, name="mean")
                    nc.vector.tensor_tensor(
                        out=mean[:, :], in0=g1, in1=invc_t[:, :], op=MULT
                    )
                    var = smpool.tile([128, S], F32, name="var")
                    # ex2 = g2*invc ; var = ex2 - mean*mean
                    nc.vector.tensor_tensor(
                        out=var[:, :], in0=g2, in1=invc_t[:, :], op=MULT
                    )
                    msq = smpool.tile([128, S], F32, name="msq")
                    nc.vector.scalar_tensor_tensor(
                        out=msq[:, :],
                        in0=mean[:, :],
                        scalar=1.0,
                        in1=mean[:, :],
                        op0=MULT,
                        op1=MULT,
                    )
                    nc.vector.tensor_tensor(
                        out=var[:, :], in0=var[:, :], in1=msq[:, :], op=SUB
                    )
                    nc.vector.tensor_scalar_add(var[:, :], var[:, :], EPS)
                    std = smpool.tile([128, S], F32, name="std")
                    nc.scalar.activation(std[:, :], var[:, :], SQRT)
                    rinv = smpool.tile([128, S], F32, name="rinv")
                    nc.vector.reciprocal(rinv[:, :], std[:, :])
                    scale = smpool.tile([128, S], F32, name="scale")
                    nc.vector.tensor_tensor(
                        out=scale[:, :], in0=rinv[:, :], in1=gcm_t[s_i][ci][:, :], op=MULT
                    )
                    shift = smpool.tile([128, S], F32, name="shift")
                    nc.vector.scalar_tensor_tensor(
                        out=shift[:, :],
                        in0=mean[:, :],
                        scalar=1.0,
                        in1=scale[:, :],
                        op0=MULT,
                        op1=MULT,
                    )
                    nc.vector.tensor_tensor(
                        out=shift[:, :],
                        in0=becm_t[s_i][ci][:, :],
                        in1=shift[:, :],
                        op=SUB,
                    )

                    # expand subject -> per-item columns via masks
                    sct = scpool.tile([128, BSH], F32, name=f"SC{s_i}_{ci}")
                    sht = scpool.tile([128, BSH], F32, name=f"SH{s_i}_{ci}")
                    for dst, src in ((sct, scale), (sht, shift)):
                        prev = None
                        for s in range(S):
                            o = dst if s == S - 1 else sctpool.tile(
                                [128, BSH], F32, name="acc"
                            )
                            if prev is None:
                                nc.vector.tensor_scalar_mul(
                                    o[:, :], mask_t[s][:, :], src[:, s : s + 1]
                                )
                            else:
                                nc.vector.scalar_tensor_tensor(
                                    out=o[:, :],
                                    in0=mask_t[s][:, :],
                                    scalar=src[:, s : s + 1],
                                    in1=prev[:, :],
                                    op0=MULT,
                                    op1=ADD,
                                )
                            prev = o
                    SCs.append(sct)
                    SHs.append(sht)
                SC[s_i] = SCs
                SH[s_i] = SHs

            conv_stage(0)
            conv_stage(1)
            conv_stage(2)

            # ---------------- final apply: out = gelu(bn2(y2)) ----------------
            for b in range(BSH):
                for ci, (c0, c1) in enumerate(CT):
                    m = c1 - c0
                    yin = yinpool.tile([128, T], F32, name="yin")
                    nc.sync.dma_start(yin[:m, :], Yd[2][b, c0:c1, :])
                    zo = ynpool.tile([128, T], F32, name="ynew")
                    nc.scalar.activation(
                        zo[:m, :],
                        yin[:m, :],
                        GELU,
                        bias=SH[2][ci][:m, b : b + 1],
                        scale=SC[2][ci][:m, b : b + 1],
                    )
                    nc.sync.dma_start(OUTd[b, c0:c1, :], zo[:m, :])

    _split_multi_waits(nc, mybir)
    return nc


_CACHED = {}


def kernel(**inputs):
    X = np.ascontiguousarray(np.asarray(inputs["X"], dtype=np.float32))
    subj = np.asarray(inputs["subject_idxs"], dtype=np.int32)
    w = [np.asarray(inputs[f"w{i}"], dtype=np.float32) for i in range(3)]
    g = [np.asarray(inputs[k], dtype=np.float32) for k in ("g0", "g1", "g2")]
    be = [np.asarray(inputs[k], dtype=np.float32) for k in ("be0", "be1", "be2")]
    # biases cancel inside per-subject BN (uniform per-channel shift is
    # absorbed by the per-subject mean), so b0/b1/b2 are not needed.

    from concourse.bass_utils import run_bass_kernel_spmd

    trace = bool(int(os.environ.get("BASS_KERNEL_TRACE", "0")))
    if trace:
        _install_ntff_hook()

    if "nc" not in _CACHED:
        _CACHED["nc"] = _build_program()
    nc = _CACHED["nc"]

    # ---------------- host-side prep ----------------
    cnt = np.maximum(
        np.bincount(subj, minlength=S).astype(np.float32) * float(T), 1.0
    )
    invc = np.broadcast_to((1.0 / cnt)[None, :], (128, S)).copy()

    shared = {"invc": invc}
    for s_i in range(3):
        for tap in range(3):
            shared[f"w{s_i}t{tap}"] = np.ascontiguousarray(w[s_i][:, :, tap].T)
        shared[f"gcm{s_i}"] = np.ascontiguousarray(g[s_i].T)  # [COUT, S]
        shared[f"becm{s_i}"] = np.ascontiguousarray(be[s_i].T)

    in_maps = []
    for c in range(NCORES):
        sl = slice(c * BSH, (c + 1) * BSH)
        subj_c = subj[sl]
        masks = np.zeros((S, 128, BSH), dtype=np.float32)
        for bi in range(BSH):
            masks[subj_c[bi], :, bi] = 1.0
        m = dict(shared)
        m["xsh"] = X[sl]
        m["masks"] = masks
        in_maps.append(m)

    res = run_bass_kernel_spmd(
        nc, in_maps, core_ids=list(range(NCORES)), trace=trace
    )
    if trace:
        _CACHED["exec_time_ns"] = res.exec_time_ns
        _CACHED["results_obj"] = res

    out = np.empty((B, COUT, T), dtype=np.float32)
    for c in range(NCORES):
        out[c * BSH : (c + 1) * BSH] = res.results[c]["out"]
    return out
